# revision 1
# baseline (speedup 1.0000x reference)
"""Trainium2 Bass kernel for nn_AttentionLayer (B=4, S=2048, H=16, DH=64).

Sharding: 8 cores = 4 batches x 2 head-groups (8 heads each). Each core
computes full attention for its (batch, head-group) shard; no cross-core
communication. The host pre-transposes/casts inputs, and post-normalizes
(softmax denominator division), transposes back, and adds the value bias.

Device dataflow per core (all matmuls bf16, PSUM f32):
  qlT[d',s] = wq_sl.T-contract  (lhsT=wq tiles [d,128], rhs=qT [d,s])
  klT[d',s] likewise; vl[j,dh'] natural (lhsT=vT tiles [d,j], rhs=wv)
  scoresT[j,i] = sum_dh klT[dh,j]*qlT[dh,i]   (K=64, head pairs row-packed)
  E = exp(0.125*scoresT)  (ACT, PSUM->SBUF bf16)
  E *= maskT              (DVE, multiplicative mask == additive -10000)
  ctxUT[dh,i] += vl_aug[j,dh].T @ E[j,i]  (vl_aug has a ones column ->
                                           row 64 = softmax denominator)
Output: [520, 2048] f32 = 8 heads x (64 ctxUT rows) + 8 den rows.
"""

import numpy as np
import ml_dtypes

import concourse.bass as bass
import concourse.mybir as mybir
import concourse.tile as tile
from concourse import bacc
from concourse.bass_utils import run_bass_kernel_spmd

BF16 = mybir.dt.bfloat16
F32 = mybir.dt.float32

S = 2048      # sequence length
D = 1024      # model dim
DL = 512      # local d' (8 heads x 64)
DH = 64       # head dim
HL = 8        # local heads
KT = 8        # k-tiles over D
MT = 4        # m-tiles over DL (128 each)
SB = 4        # s blocks of 512
JT = 16       # j tiles of 128
IB = 4        # i blocks of 512

_GRAPH = None


def build_graph():
    nc = bacc.Bacc("TRN2", target_bir_lowering=False, debug=False)

    qT = nc.dram_tensor("qT", [D, S], BF16, kind="ExternalInput").ap()
    kT = nc.dram_tensor("kT", [D, S], BF16, kind="ExternalInput").ap()
    vT = nc.dram_tensor("vT", [D, S], BF16, kind="ExternalInput").ap()
    maskT = nc.dram_tensor("maskT", [S, S], BF16, kind="ExternalInput").ap()
    wq = nc.dram_tensor("wq", [D, DL], BF16, kind="ExternalInput").ap()
    wk = nc.dram_tensor("wk", [D, DL], BF16, kind="ExternalInput").ap()
    wv = nc.dram_tensor("wv", [D, DL], BF16, kind="ExternalInput").ap()
    bq = nc.dram_tensor("bq", [DL], F32, kind="ExternalInput").ap()
    bk = nc.dram_tensor("bk", [DL], F32, kind="ExternalInput").ap()
    out = nc.dram_tensor("out", [DL + HL, S], F32, kind="ExternalOutput").ap()

    with tile.TileContext(nc) as tc:
        _build_body(tc, nc, qT, kT, vT, maskT, wq, wk, wv, bq, bk, out)

    nc.compile()
    return nc


def _build_body(tc, nc, qT, kT, vT, maskT, wq, wk, wv, bq, bk, out):
    from contextlib import ExitStack

    with ExitStack() as ctx:
        const = ctx.enter_context(tc.tile_pool(name="const", bufs=1))
        acts = ctx.enter_context(tc.tile_pool(name="acts", bufs=1))
        qk_pool = ctx.enter_context(tc.tile_pool(name="qk", bufs=2))
        e_pool = ctx.enter_context(tc.tile_pool(name="epool", bufs=4))
        m_pool = ctx.enter_context(tc.tile_pool(name="mpool", bufs=4))
        o_pool = ctx.enter_context(tc.tile_pool(name="opool", bufs=4))

        # ---- weights / biases / persistent activations ----
        wq_sb = const.tile([128, KT, DL], BF16)
        wk_sb = const.tile([128, KT, DL], BF16)
        wv_sb = const.tile([128, KT, DL], BF16)
        nc.sync.dma_start(out=wq_sb[:], in_=wq.rearrange("(kt p) n -> p kt n", p=128))
        nc.sync.dma_start(out=wk_sb[:], in_=wk.rearrange("(kt p) n -> p kt n", p=128))
        nc.sync.dma_start(out=wv_sb[:], in_=wv.rearrange("(kt p) n -> p kt n", p=128))
        bq_sb = const.tile([128, MT], F32)
        bk_sb = const.tile([128, MT], F32)
        nc.sync.dma_start(out=bq_sb[:], in_=bq.rearrange("(m p) -> p m", p=128))
        nc.sync.dma_start(out=bk_sb[:], in_=bk.rearrange("(m p) -> p m", p=128))
        zero_b = const.tile([128, 1], F32)
        nc.vector.memset(zero_b[:], 0.0)

        qlT_sb = acts.tile([128, MT, S], BF16)   # [d' partition, m-tile, s]
        klT_sb = acts.tile([128, MT, S], BF16)
        vl_sb = acts.tile([128, JT, HL, DH + 1], BF16)  # per j-tile, per head, +ones
        nc.vector.memset(vl_sb[:, :, :, DH], 1.0)

        # ---- phase 1: projections ----
        with tc.tile_pool(name="ppsum", bufs=2, space="PSUM") as ppsum:
            for sb in range(SB):
                ssl = slice(sb * 512, (sb + 1) * 512)
                qt = qk_pool.tile([128, KT, 512], BF16, tag="qt")
                kt_ = qk_pool.tile([128, KT, 512], BF16, tag="kt")
                vt = qk_pool.tile([128, KT, 512], BF16, tag="vt")
                nc.sync.dma_start(
                    out=qt[:], in_=qT[:, ssl].rearrange("(kt p) n -> p kt n", p=128))
                nc.sync.dma_start(
                    out=kt_[:], in_=kT[:, ssl].rearrange("(kt p) n -> p kt n", p=128))
                nc.sync.dma_start(
                    out=vt[:], in_=vT[:, ssl].rearrange("(kt p) n -> p kt n", p=128))
                for m in range(MT):
                    msl = slice(m * 128, (m + 1) * 128)
                    psq = ppsum.tile([128, 512], F32, tag="pq")
                    psk = ppsum.tile([128, 512], F32, tag="pk")
                    for kk in range(KT):
                        nc.tensor.matmul(
                            psq[:], wq_sb[:, kk, msl], qt[:, kk, :],
                            start=(kk == 0), stop=(kk == KT - 1))
                    for kk in range(KT):
                        nc.tensor.matmul(
                            psk[:], wk_sb[:, kk, msl], kt_[:, kk, :],
                            start=(kk == 0), stop=(kk == KT - 1))
                    nc.vector.tensor_scalar_add(
                        qlT_sb[:, m, ssl], psq[:], bq_sb[:, m:m + 1])
                    nc.vector.tensor_scalar_add(
                        klT_sb[:, m, ssl], psk[:], bk_sb[:, m:m + 1])
                for jj in range(MT):
                    jt = sb * 4 + jj
                    jsl = slice(jj * 128, (jj + 1) * 128)
                    psv = ppsum.tile([128, 512], F32, tag="pv")
                    for kk in range(KT):
                        nc.tensor.matmul(
                            psv[:], vt[:, kk, jsl], wv_sb[:, kk, :],
                            start=(kk == 0), stop=(kk == KT - 1))
                    nc.vector.tensor_copy(
                        vl_sb[:, jt, :, 0:DH],
                        psv[:].rearrange("p (h d) -> p h d", h=HL))

        # ---- phase 2: attention ----
        with (
            tc.tile_pool(name="spsum", bufs=2, space="PSUM") as spsum,
            tc.tile_pool(name="cpsum", bufs=2, space="PSUM") as cpsum,
        ):
            for hp in range(4):
                h0, h1 = 2 * hp, 2 * hp + 1
                for ib in range(IB):
                    isl = slice(ib * 512, (ib + 1) * 512)
                    ctx0 = cpsum.tile([DH + 1, 512], F32, tag="c0")
                    ctx1 = cpsum.tile([DH + 1, 512], F32, tag="c1")
                    for jt in range(JT):
                        jsl = slice(jt * 128, (jt + 1) * 128)
                        msk = m_pool.tile([128, 512], BF16, tag="msk")
                        nc.sync.dma_start(out=msk[:], in_=maskT[jsl, isl])
                        sc = spsum.tile([128, 1024], F32, tag="sc")
                        nc.tensor.matmul(
                            sc[:, 0:512],
                            klT_sb[0:64, hp, jsl], qlT_sb[0:64, hp, isl],
                            start=True, stop=True)
                        nc.tensor.matmul(
                            sc[:, 512:1024],
                            klT_sb[64:128, hp, jsl], qlT_sb[64:128, hp, isl],
                            start=True, stop=True)
                        E = e_pool.tile([128, 1024], BF16, tag="E")
                        nc.scalar.activation(
                            E[:], sc[:], mybir.ActivationFunctionType.Exp,
                            bias=zero_b[:], scale=0.125)
                        ev = E[:].rearrange("p (o n) -> p o n", o=2)
                        mb = msk[:].rearrange("p (o n) -> p o n", o=1)
                        mb = mb.broadcast_to([128, 2, 512])
                        nc.vector.tensor_tensor(
                            ev, ev, mb, mybir.AluOpType.mult)
                        nc.tensor.matmul(
                            ctx0[:], vl_sb[:, jt, h0, :], E[:, 0:512],
                            start=(jt == 0), stop=(jt == JT - 1))
                        nc.tensor.matmul(
                            ctx1[:], vl_sb[:, jt, h1, :], E[:, 512:1024],
                            start=(jt == 0), stop=(jt == JT - 1))
                    o0 = o_pool.tile([DH + 1, 512], F32, tag="o")
                    o1 = o_pool.tile([DH + 1, 512], F32, tag="o")
                    nc.vector.tensor_copy(o0[:], ctx0[:])
                    nc.vector.tensor_copy(o1[:], ctx1[:])
                    nc.sync.dma_start(
                        out=out[h0 * DH:(h0 + 1) * DH, isl], in_=o0[0:DH, :])
                    nc.sync.dma_start(
                        out=out[DL + h0:DL + h0 + 1, isl], in_=o0[DH:DH + 1, :])
                    nc.sync.dma_start(
                        out=out[h1 * DH:(h1 + 1) * DH, isl], in_=o1[0:DH, :])
                    nc.sync.dma_start(
                        out=out[DL + h1:DL + h1 + 1, isl], in_=o1[DH:DH + 1, :])


def _get_graph():
    global _GRAPH
    if _GRAPH is None:
        _GRAPH = build_graph()
    return _GRAPH


def make_in_maps(q, k, v, attention_mask, wq_kernel, wq_bias, wk_kernel,
                 wk_bias, wv_kernel, wv_bias):
    bf = ml_dtypes.bfloat16
    in_maps = []
    for c in range(8):
        b, hg = divmod(c, 2)
        sl = slice(hg * DL, (hg + 1) * DL)
        in_maps.append({
            "qT": np.asarray(q[b].T, dtype=bf),
            "kT": np.asarray(k[b].T, dtype=bf),
            "vT": np.asarray(v[b].T, dtype=bf),
            "maskT": np.asarray(attention_mask[b].T, dtype=bf),
            "wq": np.asarray(wq_kernel[:, sl], dtype=bf),
            "wk": np.asarray(wk_kernel[:, sl], dtype=bf),
            "wv": np.asarray(wv_kernel[:, sl], dtype=bf),
            "bq": np.asarray(wq_bias[sl], dtype=np.float32),
            "bk": np.asarray(wk_bias[sl], dtype=np.float32),
        })
    return in_maps


def assemble_output(results, wv_bias):
    B = 4
    out_full = np.empty((B, S, D), dtype=np.float32)
    for c in range(8):
        b, hg = divmod(c, 2)
        o = results[c]["out"]                      # [520, 2048]
        ctxUT = o[:DL].reshape(HL, DH, S)
        den = o[DL:DL + HL]                        # [8, 2048]
        ctxn = ctxUT / den[:, None, :]
        out_full[b, :, hg * DL:(hg + 1) * DL] = (
            ctxn.transpose(2, 0, 1).reshape(S, DL))
    out_full += np.asarray(wv_bias, dtype=np.float32)[None, None, :]
    return out_full


def kernel(q, k, v, attention_mask, wq_kernel, wq_bias, wk_kernel, wk_bias,
           wv_kernel, wv_bias):
    nc = _get_graph()
    in_maps = make_in_maps(q, k, v, attention_mask, wq_kernel, wq_bias,
                           wk_kernel, wk_bias, wv_kernel, wv_bias)
    res = run_bass_kernel_spmd(nc, in_maps, core_ids=list(range(8)))
    return assemble_output(res.results, wv_bias)


# revision 7
# speedup vs baseline: 1.2829x; 1.2829x over previous
"""Trainium2 Bass kernel for nn_AttentionLayer (B=4, S=2048, H=16, DH=64).

Sharding: 8 cores = 4 batches x 2 head-groups (8 heads each). Each core
computes full attention for its (batch, head-group) shard; no cross-core
communication. The host pre-transposes/casts inputs, and post-normalizes
(softmax denominator division), transposes back, and adds the value bias.

Device dataflow per core (all matmuls bf16, PSUM f32):
  qlT[d',s] = wq_sl.T-contract  (lhsT=wq tiles [d,128], rhs=qT [d,s])
  klT[d',s] likewise; vl[j,dh'] natural (lhsT=vT tiles [d,j], rhs=wv)
  scoresT[j,i] = sum_dh klT[dh,j]*qlT[dh,i]   (K=64, head pairs row-packed)
  E = exp(0.125*scoresT)  (ACT, PSUM->SBUF bf16)
  E *= maskT              (DVE, multiplicative mask == additive -10000)
  ctxUT[dh,i] += vl_aug[j,dh].T @ E[j,i]  (vl_aug has a ones column ->
                                           row 64 = softmax denominator)
Output: [520, 2048] f32 = 8 heads x (64 ctxUT rows) + 8 den rows.
"""

import numpy as np
import ml_dtypes

import concourse.bass as bass
import concourse.mybir as mybir
import concourse.tile as tile
from concourse import bacc
from concourse.bass_utils import run_bass_kernel_spmd

BF16 = mybir.dt.bfloat16
F32 = mybir.dt.float32

S = 2048      # sequence length
D = 1024      # model dim
DL = 512      # local d' (8 heads x 64)
DH = 64       # head dim
HL = 8        # local heads
KT = 8        # k-tiles over D
MT = 4        # m-tiles over DL (128 each)
SB = 4        # s blocks of 512
JT = 16       # j tiles of 128
IB = 4        # i blocks of 512

_GRAPH = None


def build_graph():
    nc = bacc.Bacc("TRN2", target_bir_lowering=False, debug=False)

    qT = nc.dram_tensor("qT", [D, S], BF16, kind="ExternalInput").ap()
    kT = nc.dram_tensor("kT", [D, S], BF16, kind="ExternalInput").ap()
    vT = nc.dram_tensor("vT", [D, S], BF16, kind="ExternalInput").ap()
    maskT = nc.dram_tensor("maskT", [S, S], BF16, kind="ExternalInput").ap()
    wq = nc.dram_tensor("wq", [D, DL], BF16, kind="ExternalInput").ap()
    wk = nc.dram_tensor("wk", [D, DL], BF16, kind="ExternalInput").ap()
    wv = nc.dram_tensor("wv", [D, DL], BF16, kind="ExternalInput").ap()
    bq = nc.dram_tensor("bq", [DL], F32, kind="ExternalInput").ap()
    bk = nc.dram_tensor("bk", [DL], F32, kind="ExternalInput").ap()
    out = nc.dram_tensor("out", [DL + HL, S], F32, kind="ExternalOutput").ap()

    with tile.TileContext(nc) as tc:
        _build_body(tc, nc, qT, kT, vT, maskT, wq, wk, wv, bq, bk, out)

    nc.compile()
    return nc


def _build_body(tc, nc, qT, kT, vT, maskT, wq, wk, wv, bq, bk, out):
    from contextlib import ExitStack

    with ExitStack() as ctx:
        const = ctx.enter_context(tc.tile_pool(name="const", bufs=1))
        acts = ctx.enter_context(tc.tile_pool(name="acts", bufs=1))
        qk_pool = ctx.enter_context(tc.tile_pool(name="qk", bufs=3))
        e_pool = ctx.enter_context(tc.tile_pool(name="epool", bufs=6))
        m_pool = ctx.enter_context(tc.tile_pool(name="mpool", bufs=8))
        o_pool = ctx.enter_context(tc.tile_pool(name="opool", bufs=4))

        # ---- weights / biases / persistent activations ----
        wq_sb = const.tile([128, KT, DL], BF16)
        wk_sb = const.tile([128, KT, DL], BF16)
        wv_sb = const.tile([128, KT, DL], BF16)
        nc.sync.dma_start(out=wq_sb[:], in_=wq.rearrange("(kt p) n -> p kt n", p=128))
        nc.scalar.dma_start(out=wk_sb[:], in_=wk.rearrange("(kt p) n -> p kt n", p=128))
        nc.sync.dma_start(out=wv_sb[:], in_=wv.rearrange("(kt p) n -> p kt n", p=128))
        bq_sb = const.tile([128, MT], F32)
        bk_sb = const.tile([128, MT], F32)
        nc.sync.dma_start(out=bq_sb[:], in_=bq.rearrange("(m p) -> p m", p=128))
        nc.sync.dma_start(out=bk_sb[:], in_=bk.rearrange("(m p) -> p m", p=128))
        zero_b = const.tile([128, 1], F32)
        nc.vector.memset(zero_b[:], 0.0)

        qlT_sb = acts.tile([128, MT, S], BF16)   # [d' partition, m-tile, s]
        klT_sb = acts.tile([128, MT, S], BF16)
        vl_sb = acts.tile([128, JT, HL, DH + 1], BF16)  # per j-tile, per head, +ones
        nc.vector.memset(vl_sb[:, :, :, DH], 1.0)

        # ---- phase 1: projections ----
        with tc.tile_pool(name="ppsum", bufs=2, space="PSUM") as ppsum:
            for sb in range(SB):
                ssl = slice(sb * 512, (sb + 1) * 512)
                qt = qk_pool.tile([128, KT, 512], BF16, tag="qt")
                kt_ = qk_pool.tile([128, KT, 512], BF16, tag="kt")
                vt = qk_pool.tile([128, KT, 512], BF16, tag="vt")
                nc.sync.dma_start(
                    out=qt[:], in_=qT[:, ssl].rearrange("(kt p) n -> p kt n", p=128))
                nc.scalar.dma_start(
                    out=kt_[:], in_=kT[:, ssl].rearrange("(kt p) n -> p kt n", p=128))
                nc.scalar.dma_start(
                    out=vt[:], in_=vT[:, ssl].rearrange("(kt p) n -> p kt n", p=128))
                for m in range(MT):
                    msl = slice(m * 128, (m + 1) * 128)
                    psq = ppsum.tile([128, 512], F32, tag="pq")
                    psk = ppsum.tile([128, 512], F32, tag="pk")
                    for kk in range(KT):
                        nc.tensor.matmul(
                            psq[:], wq_sb[:, kk, msl], qt[:, kk, :],
                            start=(kk == 0), stop=(kk == KT - 1))
                    for kk in range(KT):
                        nc.tensor.matmul(
                            psk[:], wk_sb[:, kk, msl], kt_[:, kk, :],
                            start=(kk == 0), stop=(kk == KT - 1))
                    nc.vector.tensor_scalar_add(
                        qlT_sb[:, m, ssl], psq[:], bq_sb[:, m:m + 1])
                    nc.vector.tensor_scalar_add(
                        klT_sb[:, m, ssl], psk[:], bk_sb[:, m:m + 1])
                for jj in range(MT):
                    jt = sb * 4 + jj
                    jsl = slice(jj * 128, (jj + 1) * 128)
                    psv = ppsum.tile([128, 512], F32, tag="pv")
                    for kk in range(KT):
                        nc.tensor.matmul(
                            psv[:], vt[:, kk, jsl], wv_sb[:, kk, :],
                            start=(kk == 0), stop=(kk == KT - 1))
                    nc.vector.tensor_copy(
                        vl_sb[:, jt, :, 0:DH],
                        psv[:].rearrange("p (h d) -> p h d", h=HL))

        # ---- phase 2: attention ----
        with (
            tc.tile_pool(name="spsum", bufs=2, space="PSUM") as spsum,
            tc.tile_pool(name="cpsum", bufs=2, space="PSUM") as cpsum,
        ):
            for hp in range(4):
                h0, h1 = 2 * hp, 2 * hp + 1
                for ib in range(IB):
                    isl = slice(ib * 512, (ib + 1) * 512)
                    ctx0 = cpsum.tile([DH + 1, 512], F32, tag="c0")
                    ctx1 = cpsum.tile([DH + 1, 512], F32, tag="c1")
                    for jt in range(JT):
                        jsl = slice(jt * 128, (jt + 1) * 128)
                        msk = m_pool.tile([128, 512], BF16, tag="msk")
                        nc.sync.dma_start(out=msk[:], in_=maskT[jsl, isl])
                        sc = spsum.tile([128, 1024], F32, tag="sc")
                        nc.tensor.matmul(
                            sc[:, 0:512],
                            klT_sb[0:64, hp, jsl], qlT_sb[0:64, hp, isl],
                            start=True, stop=True)
                        nc.tensor.matmul(
                            sc[:, 512:1024],
                            klT_sb[64:128, hp, jsl], qlT_sb[64:128, hp, isl],
                            start=True, stop=True)
                        E = e_pool.tile([128, 1024], BF16, tag="E")
                        nc.scalar.activation(
                            E[:], sc[:], mybir.ActivationFunctionType.Exp,
                            bias=zero_b[:], scale=0.125)
                        ev = E[:].rearrange("p (o n) -> p o n", o=2)
                        mb = msk[:].rearrange("p (o n) -> p o n", o=1)
                        mb = mb.broadcast_to([128, 2, 512])
                        nc.vector.tensor_tensor(
                            ev, ev, mb, mybir.AluOpType.mult)
                        nc.tensor.matmul(
                            ctx0[:], vl_sb[:, jt, h0, :], E[:, 0:512],
                            start=(jt == 0), stop=(jt == JT - 1))
                        nc.tensor.matmul(
                            ctx1[:], vl_sb[:, jt, h1, :], E[:, 512:1024],
                            start=(jt == 0), stop=(jt == JT - 1))
                    o0 = o_pool.tile([DH + 1, 512], F32, tag="o")
                    o1 = o_pool.tile([DH + 1, 512], F32, tag="o")
                    nc.vector.tensor_copy(o0[:], ctx0[:])
                    nc.vector.tensor_copy(o1[:], ctx1[:])
                    nc.sync.dma_start(
                        out=out[h0 * DH:(h0 + 1) * DH, isl], in_=o0[0:DH, :])
                    nc.sync.dma_start(
                        out=out[DL + h0:DL + h0 + 1, isl], in_=o0[DH:DH + 1, :])
                    nc.sync.dma_start(
                        out=out[h1 * DH:(h1 + 1) * DH, isl], in_=o1[0:DH, :])
                    nc.sync.dma_start(
                        out=out[DL + h1:DL + h1 + 1, isl], in_=o1[DH:DH + 1, :])


def _get_graph():
    global _GRAPH
    if _GRAPH is None:
        _GRAPH = build_graph()
    return _GRAPH


def make_in_maps(q, k, v, attention_mask, wq_kernel, wq_bias, wk_kernel,
                 wk_bias, wv_kernel, wv_bias):
    bf = ml_dtypes.bfloat16
    in_maps = []
    for c in range(8):
        b, hg = divmod(c, 2)
        sl = slice(hg * DL, (hg + 1) * DL)
        in_maps.append({
            "qT": np.asarray(q[b].T, dtype=bf),
            "kT": np.asarray(k[b].T, dtype=bf),
            "vT": np.asarray(v[b].T, dtype=bf),
            "maskT": np.asarray(attention_mask[b].T, dtype=bf),
            "wq": np.asarray(wq_kernel[:, sl], dtype=bf),
            "wk": np.asarray(wk_kernel[:, sl], dtype=bf),
            "wv": np.asarray(wv_kernel[:, sl], dtype=bf),
            "bq": np.asarray(wq_bias[sl], dtype=np.float32),
            "bk": np.asarray(wk_bias[sl], dtype=np.float32),
        })
    return in_maps


def assemble_output(results, wv_bias):
    B = 4
    out_full = np.empty((B, S, D), dtype=np.float32)
    for c in range(8):
        b, hg = divmod(c, 2)
        o = results[c]["out"]                      # [520, 2048]
        ctxUT = o[:DL].reshape(HL, DH, S)
        den = o[DL:DL + HL]                        # [8, 2048]
        ctxn = ctxUT / den[:, None, :]
        out_full[b, :, hg * DL:(hg + 1) * DL] = (
            ctxn.transpose(2, 0, 1).reshape(S, DL))
    out_full += np.asarray(wv_bias, dtype=np.float32)[None, None, :]
    return out_full


def kernel(q, k, v, attention_mask, wq_kernel, wq_bias, wk_kernel, wk_bias,
           wv_kernel, wv_bias):
    nc = _get_graph()
    in_maps = make_in_maps(q, k, v, attention_mask, wq_kernel, wq_bias,
                           wk_kernel, wk_bias, wv_kernel, wv_bias)
    res = run_bass_kernel_spmd(nc, in_maps, core_ids=list(range(8)))
    return assemble_output(res.results, wv_bias)


# revision 10
# speedup vs baseline: 1.2840x; 1.0008x over previous
"""Trainium2 Bass kernel for nn_AttentionLayer (B=4, S=2048, H=16, DH=64).

Sharding: 8 cores = 4 batches x 2 head-groups (8 heads each). Each core
computes full attention for its (batch, head-group) shard; no cross-core
communication. The host pre-transposes/casts inputs, and post-normalizes
(softmax denominator division), transposes back, and adds the value bias.

Device dataflow per core (all matmuls bf16, PSUM f32):
  qlT[d',s] = wq_sl.T-contract  (lhsT=wq tiles [d,128], rhs=qT [d,s])
  klT[d',s] likewise; vl[j,dh'] natural (lhsT=vT tiles [d,j], rhs=wv)
  scoresT[j,i] = sum_dh klT[dh,j]*qlT[dh,i]   (K=64, head pairs row-packed)
  E = exp(0.125*scoresT)  (ACT, PSUM->SBUF bf16)
  E *= maskT              (DVE, multiplicative mask == additive -10000)
  ctxUT[dh,i] += vl_aug[j,dh].T @ E[j,i]  (vl_aug has a ones column ->
                                           row 64 = softmax denominator)
Output: [520, 2048] f32 = 8 heads x (64 ctxUT rows) + 8 den rows.
"""

import numpy as np
import ml_dtypes

import concourse.bass as bass
import concourse.mybir as mybir
import concourse.tile as tile
from concourse import bacc
from concourse.bass_utils import run_bass_kernel_spmd

BF16 = mybir.dt.bfloat16
F32 = mybir.dt.float32

S = 2048      # sequence length
D = 1024      # model dim
DL = 512      # local d' (8 heads x 64)
DH = 64       # head dim
HL = 8        # local heads
KT = 8        # k-tiles over D
MT = 4        # m-tiles over DL (128 each)
SB = 4        # s blocks of 512
JT = 16       # j tiles of 128
IB = 4        # i blocks of 512

_GRAPH = None


def build_graph():
    nc = bacc.Bacc("TRN2", target_bir_lowering=False, debug=False)

    qT = nc.dram_tensor("qT", [D, S], BF16, kind="ExternalInput").ap()
    kT = nc.dram_tensor("kT", [D, S], BF16, kind="ExternalInput").ap()
    vT = nc.dram_tensor("vT", [D, S], BF16, kind="ExternalInput").ap()
    maskT = nc.dram_tensor("maskT", [S, S], BF16, kind="ExternalInput").ap()
    wq = nc.dram_tensor("wq", [D, DL], BF16, kind="ExternalInput").ap()
    wk = nc.dram_tensor("wk", [D, DL], BF16, kind="ExternalInput").ap()
    wv = nc.dram_tensor("wv", [D, DL], BF16, kind="ExternalInput").ap()
    bq = nc.dram_tensor("bq", [DL], F32, kind="ExternalInput").ap()
    bk = nc.dram_tensor("bk", [DL], F32, kind="ExternalInput").ap()
    out = nc.dram_tensor("out", [DL + HL, S], F32, kind="ExternalOutput").ap()

    with tile.TileContext(nc) as tc:
        _build_body(tc, nc, qT, kT, vT, maskT, wq, wk, wv, bq, bk, out)

    nc.compile()
    return nc


def _build_body(tc, nc, qT, kT, vT, maskT, wq, wk, wv, bq, bk, out):
    from contextlib import ExitStack

    with ExitStack() as ctx:
        const = ctx.enter_context(tc.tile_pool(name="const", bufs=1))
        acts = ctx.enter_context(tc.tile_pool(name="acts", bufs=1))
        qk_pool = ctx.enter_context(tc.tile_pool(name="qk", bufs=3))
        e_pool = ctx.enter_context(tc.tile_pool(name="epool", bufs=8))
        m_pool = ctx.enter_context(tc.tile_pool(name="mpool", bufs=12))
        o_pool = ctx.enter_context(tc.tile_pool(name="opool", bufs=6))

        # ---- weights / biases / persistent activations ----
        wq_sb = const.tile([128, KT, DL], BF16)
        wk_sb = const.tile([128, KT, DL], BF16)
        wv_sb = const.tile([128, KT, DL], BF16)
        nc.sync.dma_start(out=wq_sb[:], in_=wq.rearrange("(kt p) n -> p kt n", p=128))
        nc.scalar.dma_start(out=wk_sb[:], in_=wk.rearrange("(kt p) n -> p kt n", p=128))
        nc.sync.dma_start(out=wv_sb[:], in_=wv.rearrange("(kt p) n -> p kt n", p=128))
        bq_sb = const.tile([128, MT], F32)
        bk_sb = const.tile([128, MT], F32)
        nc.sync.dma_start(out=bq_sb[:], in_=bq.rearrange("(m p) -> p m", p=128))
        nc.sync.dma_start(out=bk_sb[:], in_=bk.rearrange("(m p) -> p m", p=128))
        zero_b = const.tile([128, 1], F32)
        nc.vector.memset(zero_b[:], 0.0)

        qlT_sb = acts.tile([128, MT, S], BF16)   # [d' partition, m-tile, s]
        klT_sb = acts.tile([128, MT, S], BF16)
        vl_sb = acts.tile([128, JT, HL, DH + 1], BF16)  # per j-tile, per head, +ones
        nc.vector.memset(vl_sb[:, :, :, DH], 1.0)

        # ---- phase 1: projections ----
        with tc.tile_pool(name="ppsum", bufs=2, space="PSUM") as ppsum:
            for sb in range(SB):
                ssl = slice(sb * 512, (sb + 1) * 512)
                qt = qk_pool.tile([128, KT, 512], BF16, tag="qt")
                kt_ = qk_pool.tile([128, KT, 512], BF16, tag="kt")
                vt = qk_pool.tile([128, KT, 512], BF16, tag="vt")
                nc.sync.dma_start(
                    out=qt[:], in_=qT[:, ssl].rearrange("(kt p) n -> p kt n", p=128))
                nc.scalar.dma_start(
                    out=kt_[:], in_=kT[:, ssl].rearrange("(kt p) n -> p kt n", p=128))
                nc.scalar.dma_start(
                    out=vt[:], in_=vT[:, ssl].rearrange("(kt p) n -> p kt n", p=128))
                for m in range(MT):
                    msl = slice(m * 128, (m + 1) * 128)
                    psq = ppsum.tile([128, 512], F32, tag="pq")
                    psk = ppsum.tile([128, 512], F32, tag="pk")
                    for kk in range(KT):
                        nc.tensor.matmul(
                            psq[:], wq_sb[:, kk, msl], qt[:, kk, :],
                            start=(kk == 0), stop=(kk == KT - 1))
                    for kk in range(KT):
                        nc.tensor.matmul(
                            psk[:], wk_sb[:, kk, msl], kt_[:, kk, :],
                            start=(kk == 0), stop=(kk == KT - 1))
                    nc.vector.tensor_scalar_add(
                        qlT_sb[:, m, ssl], psq[:], bq_sb[:, m:m + 1])
                    nc.vector.tensor_scalar_add(
                        klT_sb[:, m, ssl], psk[:], bk_sb[:, m:m + 1])
                for jj in range(MT):
                    jt = sb * 4 + jj
                    jsl = slice(jj * 128, (jj + 1) * 128)
                    psv = ppsum.tile([128, 512], F32, tag="pv")
                    for kk in range(KT):
                        nc.tensor.matmul(
                            psv[:], vt[:, kk, jsl], wv_sb[:, kk, :],
                            start=(kk == 0), stop=(kk == KT - 1))
                    nc.vector.tensor_copy(
                        vl_sb[:, jt, :, 0:DH],
                        psv[:].rearrange("p (h d) -> p h d", h=HL))

        # ---- phase 2: attention ----
        with (
            tc.tile_pool(name="spsum", bufs=2, space="PSUM") as spsum,
            tc.tile_pool(name="cpsum", bufs=2, space="PSUM") as cpsum,
        ):
            for hp in range(4):
                h0, h1 = 2 * hp, 2 * hp + 1
                for ib in range(IB):
                    isl = slice(ib * 512, (ib + 1) * 512)
                    ctx0 = cpsum.tile([DH + 1, 512], F32, tag="c0")
                    ctx1 = cpsum.tile([DH + 1, 512], F32, tag="c1")
                    for jt in range(JT):
                        jsl = slice(jt * 128, (jt + 1) * 128)
                        msk = m_pool.tile([128, 512], BF16, tag="msk")
                        nc.sync.dma_start(out=msk[:], in_=maskT[jsl, isl])
                        sc = spsum.tile([128, 1024], F32, tag="sc")
                        nc.tensor.matmul(
                            sc[:, 0:512],
                            klT_sb[0:64, hp, jsl], qlT_sb[0:64, hp, isl],
                            start=True, stop=True)
                        nc.tensor.matmul(
                            sc[:, 512:1024],
                            klT_sb[64:128, hp, jsl], qlT_sb[64:128, hp, isl],
                            start=True, stop=True)
                        E = e_pool.tile([128, 1024], BF16, tag="E")
                        nc.scalar.activation(
                            E[:], sc[:], mybir.ActivationFunctionType.Exp,
                            bias=zero_b[:], scale=0.125)
                        ev = E[:].rearrange("p (o n) -> p o n", o=2)
                        mb = msk[:].rearrange("p (o n) -> p o n", o=1)
                        mb = mb.broadcast_to([128, 2, 512])
                        nc.vector.tensor_tensor(
                            ev, ev, mb, mybir.AluOpType.mult)
                        nc.tensor.matmul(
                            ctx0[:], vl_sb[:, jt, h0, :], E[:, 0:512],
                            start=(jt == 0), stop=(jt == JT - 1))
                        nc.tensor.matmul(
                            ctx1[:], vl_sb[:, jt, h1, :], E[:, 512:1024],
                            start=(jt == 0), stop=(jt == JT - 1))
                    o0 = o_pool.tile([DH + 1, 512], F32, tag="o")
                    o1 = o_pool.tile([DH + 1, 512], F32, tag="o")
                    nc.vector.tensor_copy(o0[:], ctx0[:])
                    nc.vector.tensor_copy(o1[:], ctx1[:])
                    nc.sync.dma_start(
                        out=out[h0 * DH:(h0 + 1) * DH, isl], in_=o0[0:DH, :])
                    nc.sync.dma_start(
                        out=out[DL + h0:DL + h0 + 1, isl], in_=o0[DH:DH + 1, :])
                    nc.sync.dma_start(
                        out=out[h1 * DH:(h1 + 1) * DH, isl], in_=o1[0:DH, :])
                    nc.sync.dma_start(
                        out=out[DL + h1:DL + h1 + 1, isl], in_=o1[DH:DH + 1, :])


def _get_graph():
    global _GRAPH
    if _GRAPH is None:
        _GRAPH = build_graph()
    return _GRAPH


def make_in_maps(q, k, v, attention_mask, wq_kernel, wq_bias, wk_kernel,
                 wk_bias, wv_kernel, wv_bias):
    bf = ml_dtypes.bfloat16
    in_maps = []
    for c in range(8):
        b, hg = divmod(c, 2)
        sl = slice(hg * DL, (hg + 1) * DL)
        in_maps.append({
            "qT": np.asarray(q[b].T, dtype=bf),
            "kT": np.asarray(k[b].T, dtype=bf),
            "vT": np.asarray(v[b].T, dtype=bf),
            "maskT": np.asarray(attention_mask[b].T, dtype=bf),
            "wq": np.asarray(wq_kernel[:, sl], dtype=bf),
            "wk": np.asarray(wk_kernel[:, sl], dtype=bf),
            "wv": np.asarray(wv_kernel[:, sl], dtype=bf),
            "bq": np.asarray(wq_bias[sl], dtype=np.float32),
            "bk": np.asarray(wk_bias[sl], dtype=np.float32),
        })
    return in_maps


def assemble_output(results, wv_bias):
    B = 4
    out_full = np.empty((B, S, D), dtype=np.float32)
    for c in range(8):
        b, hg = divmod(c, 2)
        o = results[c]["out"]                      # [520, 2048]
        ctxUT = o[:DL].reshape(HL, DH, S)
        den = o[DL:DL + HL]                        # [8, 2048]
        ctxn = ctxUT / den[:, None, :]
        out_full[b, :, hg * DL:(hg + 1) * DL] = (
            ctxn.transpose(2, 0, 1).reshape(S, DL))
    out_full += np.asarray(wv_bias, dtype=np.float32)[None, None, :]
    return out_full


def kernel(q, k, v, attention_mask, wq_kernel, wq_bias, wk_kernel, wk_bias,
           wv_kernel, wv_bias):
    nc = _get_graph()
    in_maps = make_in_maps(q, k, v, attention_mask, wq_kernel, wq_bias,
                           wk_kernel, wk_bias, wv_kernel, wv_bias)
    res = run_bass_kernel_spmd(nc, in_maps, core_ids=list(range(8)))
    return assemble_output(res.results, wv_bias)
